# revision 18
# baseline (speedup 1.0000x reference)
"""FFM layer (embedding lookup + field-factorization) on 8 trn2 NeuronCores.

Strategy: data-parallel over batch (4096 rows -> 512/core), one packed
lookup table replicated to every core.  The reference's inner j-sum
e[b,f,:] = sum_j v[idx[b,f], j, :] is a pure function of the table row,
so the host precomputes a 9-float payload per global feature id:

  row[gid, 0:8] = (sum_j v[gid, j, :]) / sqrt(2)              (vsum')
  row[gid, 8]   = w[gid] + w0/26 - 0.5*|sum_j v[gid, j, :]|^2 (affine)

so that  out[b] = |sum_f row[gid[b,f]][0:8]|^2 + sum_f row[gid[b,f]][8].

This shrinks each gathered row from 1 KiB (26x8 v floats) to a 512 B
padded row, cutting HBM gather traffic 4x and the VectorE reduction ~26x
versus gathering raw v rows.  Lookups use the SWDGE dma_gather custom
instruction, one per field (field-local int16 indices into the field's
20000-row subtable), 512 indices per gather.  Q7 descriptor generation
(~8.5 ns/descriptor, measured) is the bottleneck; gathers for field
group g+1 overlap the (now tiny) VectorE reduction of group g, and the
index upload is chunked per group so the first gather starts early.
"""

import sys

import numpy as np

FIELD = 26
K = 8
RPAD = 128               # padded row length in f32 (512 B)
VOCAB = 20000
TOTAL = FIELD * VOCAB    # 520000
B = 4096
NCORES = 8
BC = B // NCORES         # 512 batch rows per core
P = 128
NT = BC // P             # 4 batch tiles
NSLOT = BC // 16         # 32 int16 index slots per idx partition

# field groups for gather/compute pipelining
GROUPS = [list(range(s, min(s + 7, FIELD))) for s in range(0, FIELD, 7)]

_TRN_REPO = "/opt/trn_rl_repo"

_cache = {}


def _build_nc():
    if _TRN_REPO not in sys.path:
        sys.path.insert(0, _TRN_REPO)
    from concourse import bacc, mybir, tile

    f32 = mybir.dt.float32
    i16 = mybir.dt.int16
    Alu = mybir.AluOpType
    Ax = mybir.AxisListType

    nc = bacc.Bacc("TRN2", target_bir_lowering=False, debug=False)
    # idx16[p, f, s] = int16 field-local index of batch row s*16+(p%16),
    # field f -- 16-partition wrap replicated to 128 host-side
    idx_d = nc.dram_tensor("idx16", [P, FIELD, NSLOT], i16,
                           kind="ExternalInput")
    tab_d = nc.dram_tensor("tab", [TOTAL, RPAD], f32, kind="ExternalInput")
    out_d = nc.dram_tensor("out", [BC, 1], f32, kind="ExternalOutput")

    NG = len(GROUPS)

    with tile.TileContext(nc) as tc:
        with tc.tile_pool(name="const", bufs=1) as cpool, \
             tc.tile_pool(name="vgp", bufs=2) as vpool:
            idx_sb = cpool.tile([P, FIELD, NSLOT], i16, tag="idx")
            nc.sync.dma_start(out=idx_sb[:], in_=idx_d[:, :, :])

            # sgp[p, t, c, g] = sum over group g's fields of payload col c
            sgp = cpool.tile([P, NT, 9, NG], f32, tag="sgp")

            for gi, grp in enumerate(GROUPS):
                f0, gsz = grp[0], len(grp)
                vg = vpool.tile([P, gsz, NT, RPAD], f32, tag=f"vg{gi % 2}")
                for j, f in enumerate(grp):
                    nc.gpsimd.dma_gather(
                        out_ap=vg[:, j],
                        in_ap=tab_d[f * VOCAB:(f + 1) * VOCAB, :],
                        idxs_ap=idx_sb[:, f, :],
                        num_idxs=BC,
                        num_idxs_reg=BC,
                        elem_size=RPAD,
                    )
                # field-reduction of the 9 payload cols for this group
                nc.vector.tensor_reduce(
                    out=sgp[:, :, :, gi],
                    in_=vg[:, :, :, 0:9].rearrange("p f t c -> p t c f"),
                    axis=Ax.X,
                    op=Alu.add,
                )

            # combine groups: s16[p, t, c] = sum_g sgp[p, t, c, g]
            s16 = cpool.tile([P, NT, 9], f32, tag="s16")
            nc.vector.tensor_reduce(
                out=s16[:], in_=sgp[:], axis=Ax.X, op=Alu.add
            )
            esq = cpool.tile([P, NT, K], f32, tag="esq")
            nc.vector.tensor_tensor(
                out=esq[:], in0=s16[:, :, 0:K], in1=s16[:, :, 0:K],
                op=Alu.mult,
            )
            s2s = cpool.tile([P, NT], f32, tag="s2s")
            nc.vector.tensor_reduce(
                out=s2s[:], in_=esq[:], axis=Ax.X, op=Alu.add
            )
            out_all = cpool.tile([P, NT], f32, tag="oa")
            nc.vector.tensor_tensor(
                out=out_all[:], in0=s2s[:], in1=s16[:, :, K], op=Alu.add
            )
            # single store: out[t*128+p] = out_all[p, t]
            nc.sync.dma_start(
                out=out_d[:, :].rearrange("(t p) one -> p (t one)", p=P),
                in_=out_all[:],
            )
    nc.compile()
    return nc


def get_nc():
    if "nc" not in _cache:
        _cache["nc"] = _build_nc()
    return _cache["nc"]


def make_in_maps(inputs, offsets, w0, w, v):
    del offsets  # folded into the per-field subtable slicing
    inp = np.asarray(inputs)
    # field-local int16 indices, wrapped: idx16[f, p, s] = inputs[s*16+p, f]
    idx16 = np.ascontiguousarray(
        inp.astype(np.int16).reshape(NCORES, BC, FIELD)
    )
    v32 = np.asarray(v, dtype=np.float32).reshape(TOTAL, FIELD, K)
    vsum = v32.sum(axis=1)                                   # [TOTAL, 8]
    w0f = np.float32(np.asarray(w0, np.float32).reshape(()))
    tab = np.zeros((TOTAL, RPAD), dtype=np.float32)
    tab[:, 0:K] = vsum * np.float32(1.0 / np.sqrt(2.0))
    tab[:, K] = (
        np.asarray(w, dtype=np.float32).reshape(TOTAL)
        + w0f / np.float32(FIELD)
        - np.float32(0.5) * np.sum(vsum * vsum, axis=1)
    )
    maps = []
    for i in range(NCORES):
        shard = idx16[i]                       # [BC, FIELD]
        wrapped = shard.reshape(NSLOT, 16, FIELD).transpose(1, 2, 0)
        # [16, FIELD, NSLOT] -> replicate to 128 partitions
        rep = np.ascontiguousarray(np.tile(wrapped, (NCORES, 1, 1)))
        maps.append({"idx16": rep, "tab": tab})
    return maps


def kernel(inputs, offsets, w0, w, v):
    if _TRN_REPO not in sys.path:
        sys.path.insert(0, _TRN_REPO)
    from concourse.bass_utils import run_bass_kernel_spmd

    nc = get_nc()
    in_maps = make_in_maps(inputs, offsets, w0, w, v)
    res = run_bass_kernel_spmd(nc, in_maps, list(range(NCORES)))
    out = np.concatenate(
        [np.asarray(res.results[i]["out"]) for i in range(NCORES)], axis=0
    )
    return out.astype(np.float32)
